# revision 22
# baseline (speedup 1.0000x reference)
"""Trainium2 Bass kernel for nn_Attention_68015102099893 (sparse_attention).

B=2048 independent 9x9 attention blocks over patch tokens, fc 512->256.
Pure data parallel over 8 cores (256 batches each); on-core, 14 batches
group into 126-row tiles so per-batch 9x9 attention becomes block-diag
math driven by a host-precomputed mask.

v4 (vs the dual-load v2 baseline):
- Single input load: only the row-major xv [rows, P*C] shard is DMA'd
  (HBM traffic ~43MB/core instead of ~64MB). The channel-major xT
  layout needed by the fc matmul is built on-chip with PE transposes
  (PSUM) + vector-engine copies to SBUF.
- The second-order softmax is algebraically collapsed: with the -100
  diagonal self-mask, softmax(attn@attn/3) == I to beyond f64 precision,
  so attn2 is replaced by +1 on the diagonal folded into the mask.
- The attention matrix f = gram*scale + mask is SYMMETRIC, so with the
  softmax max-subtraction dropped (logits are O(10), exp is safe in
  f32), exp(f) is symmetric and serves directly as the lhsT of attn@v.
  No transpose of the attention weights; 1/rowsum is applied as a
  per-partition scale in the PSUM->SBUF output copies.
"""

import os
import numpy as np

PS = 3
N = 9           # patches per image
P = 9           # tokens per patch
B = 2048
C = 512
HID = 256
NCORES = 8
BLOC = B // NCORES          # 256 batches per core
ROWS = BLOC * N             # 2304 rows per core
G = 14                      # batches per group
GR = G * N                  # 126 rows per group
FULL_GROUPS = BLOC // G     # 18
REM = BLOC - FULL_GROUPS * G  # 4 remaining batches
SCALE = float((HID * P) ** -0.5)
NEG = -1.0e30

_CACHE = {}


def _groups():
    gs = [(g * G, G) for g in range(FULL_GROUPS)]
    if REM:
        gs.append((FULL_GROUPS * G, REM))
    return gs


def _build():
    import concourse.bacc as bacc
    import concourse.tile as tile
    from concourse import mybir

    BF = mybir.dt.bfloat16
    F32 = mybir.dt.float32
    I32 = mybir.dt.int32
    Copy = mybir.ActivationFunctionType.Copy
    Identity = mybir.ActivationFunctionType.Identity
    Exp = mybir.ActivationFunctionType.Exp
    MUL = mybir.AluOpType.mult
    ADD = mybir.AluOpType.add

    nc = bacc.Bacc("TRN2", target_bir_lowering=False)

    # xv rows padded on the host (+128 zero rows) so every group loads a
    # full 128-row tile (the PE transpose consumes 128 token columns).
    xv = nc.dram_tensor("xv", [ROWS + 128, P * C], BF, kind="ExternalInput")
    # pre-transposed xT4-layout blocks for p-slices 4..8 (host-prepared):
    # xtd[c0, (g, ki, j, t)] = xv_group_g[t, (4+ki)*512 + j*128 + c0]
    ngroups = len(_groups())
    xtd = nc.dram_tensor(
        "xtd", [128, ngroups * 5 * 512], BF, kind="ExternalInput"
    )
    wT = nc.dram_tensor("wT", [C, HID], BF, kind="ExternalInput")
    b2 = nc.dram_tensor("b2", [128, 2], F32, kind="ExternalInput")
    m4 = nc.dram_tensor("m4", [GR, GR], F32, kind="ExternalInput")
    eye = nc.dram_tensor("eye", [128, 128], BF, kind="ExternalInput")
    out = nc.dram_tensor("out", [ROWS, P * C], BF, kind="ExternalOutput")

    groups = _groups()
    ng = len(groups)

    with tile.TileContext(nc) as tc:
        with (
            tc.tile_pool(name="const", bufs=1) as cpool,
            tc.tile_pool(name="xvp", bufs=5) as xvpool,
            tc.tile_pool(name="xtp", bufs=3) as xtpool,
            tc.tile_pool(name="xqp", bufs=2) as xqpool,
            tc.tile_pool(name="outp", bufs=3) as outpool,
            tc.tile_pool(name="small", bufs=3) as spool,
            tc.tile_pool(name="psfc", bufs=2, space="PSUM") as psfc_pool,
            tc.tile_pool(name="psg", bufs=2, space="PSUM") as psg_pool,
            tc.tile_pool(name="pso", bufs=2, space="PSUM") as pso_pool,
            tc.tile_pool(name="psx", bufs=2, space="PSUM") as psx_pool,
        ):
            wS = cpool.tile([128, 4 * HID], BF)
            for j in range(4):
                nc.sync.dma_start(
                    out=wS[:, j * HID:(j + 1) * HID],
                    in_=wT[j * 128:(j + 1) * 128, :],
                )
            bS = cpool.tile([128, 2], F32)
            nc.sync.dma_start(out=bS[:, :], in_=b2[:, :])
            m4S = cpool.tile([GR, GR], F32)
            nc.sync.dma_start(out=m4S[:, :], in_=m4[:, :])
            eyeS = cpool.tile([128, 128], BF)
            nc.sync.dma_start(out=eyeS[:, :], in_=eye[:, :])

            st = {}

            def emit_load(g):
                b0, gb = groups[g]
                r0 = b0 * N
                xvT = xvpool.tile([128, P * C], BF, tag="xv")
                # 3 chunks so the first transposes can start ~1/3 load in
                for ck in range(3):
                    c0 = ck * 3 * 512
                    nc.sync.dma_start(
                        out=xvT[:, c0:c0 + 3 * 512],
                        in_=xv[r0:r0 + 128, c0:c0 + 3 * 512],
                    )
                st[g] = {"xvT": xvT, "rows": gb * N, "r0": r0}

            def emit_xpose_start(g):
                # allocate the destination tile; p-slices 4..8 arrive by DMA
                # (pre-transposed on host), 0..3 via PE transpose units
                s = st[g]
                # xT4[c0, p, j, t] = xv[t, p*512 + j*128 + c0]
                s["xT4"] = xtpool.tile(
                    [128, P, 4, 128], BF, tag="xt4", name="xt4"
                )
                nc.sync.dma_start(
                    out=s["xT4"][:, 4:9],
                    in_=xtd[:, g * 5 * 512:(g + 1) * 5 * 512],
                )

            def emit_xpose_unit(g, p0, np_):
                # one unit: np_ (1 or 2) p-slices transposed into one PSUM
                # bank, then a single int32-view copy to SBUF
                s = st[g]
                xvT, xT4 = s["xvT"], s["xT4"]
                px = psx_pool.tile([128, 2, 4, 128], BF, tag="psx", name="px")
                for i in range(np_):
                    p = p0 + i
                    for j in range(4):
                        nc.tensor.transpose(
                            px[:, i, j],
                            xvT[:, p * 512 + j * 128:p * 512 + (j + 1) * 128],
                            eyeS[:, :],
                        )
                nc.vector.tensor_copy(
                    out=xT4[:, p0:p0 + np_].bitcast(I32),
                    in_=px[:, 0:np_].bitcast(I32),
                )

            def emit_fc_chunk(g, h, pc):
                s = st[g]
                xT4, xq = s["xT4"], s["xq"]
                ps = psfc_pool.tile([128, 3 * 128], F32, tag="psfc")
                for j in range(4):
                    nc.tensor.matmul(
                        ps[:, :],
                        lhsT=wS[:, j * HID + h * 128:j * HID + h * 128 + 128],
                        rhs=xT4[:, 3 * pc:3 * pc + 3, j, :],
                        start=(j == 0),
                        stop=(j == 3),
                    )
                nc.scalar.activation(
                    xq[:, h, 3 * pc:3 * pc + 3, :], ps[:, :],
                    Identity, bias=bS[:, h:h + 1], scale=1.0,
                )

            def emit_gram(g):
                s = st[g]
                rows = s["rows"]
                xq = s["xq"]
                psg = psg_pool.tile([GR, GR], F32, tag="psg")
                for h in range(2):
                    for p in range(P):
                        k = h * P + p
                        sl = xq[:, h, p, :rows]
                        nc.tensor.matmul(
                            psg[:rows, :rows],
                            lhsT=sl,
                            rhs=sl,
                            start=(k == 0),
                            stop=(k == 2 * P - 1),
                        )
                s["psg"] = psg

            def emit_tail_a(g):
                s = st[g]
                rows = s["rows"]
                psg = s["psg"]
                # f = gram*scale + mask; symmetric (gram and mask both are).
                f2 = spool.tile([128, 128], F32, tag="f2")
                nc.vector.scalar_tensor_tensor(
                    out=f2[:rows, :rows], in0=psg[:rows, :rows],
                    scalar=SCALE, in1=m4S[:rows, :rows], op0=MUL, op1=ADD,
                )
                # exp without max-subtraction: in-block logits are O(10),
                # exp stays far from f32/bf16 overflow; masked entries
                # underflow to exactly 0.  e2 is symmetric -> usable as lhsT.
                e2 = spool.tile([128, 128], BF, tag="e2")
                sm2 = spool.tile([128, 1], F32, tag="sm2")
                nc.scalar.activation(
                    e2[:rows, :rows], f2[:rows, :rows], Exp,
                    bias=0.0, scale=1.0, accum_out=sm2[:rows],
                )
                ri2 = spool.tile([128, 1], F32, tag="ri2")
                nc.vector.reciprocal(ri2[:rows], sm2[:rows])
                s["e2"] = e2
                s["ri2"] = ri2

            def emit_tail_b(g):
                s = st[g]
                rows, r0 = s["rows"], s["r0"]
                e2, ri2, xvT = s["e2"], s["ri2"], s["xvT"]
                outsb = outpool.tile([GR, P * C], BF, tag="outsb")
                for p in range(P):
                    pso = pso_pool.tile([GR, 512], F32, tag="pso")
                    nc.tensor.matmul(
                        pso[:rows, :],
                        lhsT=e2[:rows, :rows],
                        rhs=xvT[:rows, p * 512:(p + 1) * 512],
                        start=True, stop=True,
                    )
                    osl = outsb[:rows, p * 512:(p + 1) * 512]
                    if p % 2 == 0 and p < 8:
                        nc.scalar.activation(
                            osl, pso[:rows, :], Copy, scale=ri2[:rows],
                        )
                    else:
                        nc.vector.tensor_scalar_mul(
                            osl, pso[:rows, :], ri2[:rows],
                        )
                nc.gpsimd.dma_start(out=out[r0:r0 + rows, :], in_=outsb[:rows, :])
                del st[g]

            XP_UNITS = [(0, 2), (2, 2)]

            def emit_xpose_all(g):
                emit_xpose_start(g)
                for p0, np_ in XP_UNITS:
                    emit_xpose_unit(g, p0, np_)

            emit_load(0)
            emit_load(1)
            emit_xpose_all(0)
            emit_load(2)
            emit_xpose_all(1)
            for g in range(ng):
                if g > 0:
                    emit_tail_a(g - 1)
                if g + 3 < ng:
                    emit_load(g + 3)
                # xq tile for g allocated up-front; fc chunks interleave with
                # the transposes of group g+2 so the PE queue never heads-of-
                # line-blocks on a psx buffer waiting for its DVE drain.
                st[g]["xq"] = xqpool.tile(
                    [128, 2, P, 128], BF, tag="xq", name="xq"
                )
                do_xp = g + 2 < ng
                if do_xp:
                    emit_xpose_start(g + 2)
                xp = 0
                for k, (h, pc) in enumerate(
                    [(h, pc) for h in range(2) for pc in range(3)]
                ):
                    if do_xp and xp < len(XP_UNITS):
                        emit_xpose_unit(g + 2, *XP_UNITS[xp])
                        xp += 1
                    emit_fc_chunk(g, h, pc)
                emit_gram(g)
                if g > 0:
                    emit_tail_b(g - 1)
            emit_tail_a(ng - 1)
            emit_tail_b(ng - 1)

    nc.finalize()
    return nc


def _host_prep(x, W_fc, b_fc):
    from concourse import mybir

    bf16 = mybir.dt.np(mybir.dt.bfloat16)
    # patch view: token order (b, n=(mi,mj), p=(pi,pj))
    xfc = x.reshape(B, PS, PS, PS, PS, C).transpose(0, 1, 3, 2, 4, 5)
    xfc = np.ascontiguousarray(xfc).reshape(B, N * P, C)

    blockmask = np.kron(np.eye(G, dtype=np.float32), np.ones((N, N), np.float32))
    # off-block -1e30; diagonal -99 = -100 (self mask) + 1 (attn2 == I)
    m4 = ((1.0 - blockmask) * NEG - 99.0 * np.eye(GR, dtype=np.float32)).astype(
        np.float32
    )
    eye = np.eye(128, dtype=np.float32).astype(bf16)
    wT = np.ascontiguousarray(W_fc.T).astype(bf16)           # [C, HID]
    b2 = np.ascontiguousarray(b_fc.reshape(2, 128).T).astype(np.float32)

    groups = _groups()
    in_maps = []
    for i in range(NCORES):
        sh = xfc[i * BLOC:(i + 1) * BLOC].reshape(BLOC * N, P * C)
        xv_i = np.zeros((ROWS + 128, P * C), dtype=bf16)
        xv_i[:ROWS] = sh.astype(bf16)
        # pre-transposed p-slices 4..8 per group, xT4 layout [c0, ki, j, t]
        xtd_i = np.empty((128, len(groups), 5, 4, 128), dtype=bf16)
        for g, (b0, gb) in enumerate(groups):
            r0 = b0 * N
            blk = xv_i[r0:r0 + 128, 4 * 512:9 * 512]       # [t, ki*512+j*128+c0]
            xtd_i[:, g] = blk.reshape(128, 5, 4, 128).transpose(3, 1, 2, 0)
        xtd_i = np.ascontiguousarray(xtd_i).reshape(128, len(groups) * 5 * 512)
        in_maps.append({
            "xv": xv_i, "xtd": xtd_i, "wT": wT, "b2": b2, "m4": m4, "eye": eye,
        })
    return in_maps


def kernel(x, W_fc, b_fc):
    from concourse.bass_utils import run_bass_kernel_spmd

    x = np.asarray(x, dtype=np.float32)
    W_fc = np.asarray(W_fc, dtype=np.float32)
    b_fc = np.asarray(b_fc, dtype=np.float32)

    if "nc" not in _CACHE:
        _CACHE["nc"] = _build()
    nc = _CACHE["nc"]
    in_maps = _host_prep(x, W_fc, b_fc)

    trace = bool(int(os.environ.get("KERNEL_TRACE", "0")))
    res = run_bass_kernel_spmd(
        nc, in_maps, core_ids=list(range(NCORES)), trace=trace
    )
    _CACHE["last_result"] = res

    outs = [np.asarray(r["out"], dtype=np.float32) for r in res.results]
    o = np.concatenate(outs, axis=0).reshape(B, PS, PS, PS, PS, C)
    o = o.transpose(0, 1, 3, 2, 4, 5).reshape(B, N, N, C)
    return np.ascontiguousarray(o)


# revision 29
# speedup vs baseline: 1.1367x; 1.1367x over previous
"""Trainium2 Bass kernel for nn_Attention_68015102099893 (sparse_attention).

B=2048 independent 9x9 attention blocks over patch tokens, fc 512->256.
Pure data parallel over 8 cores (256 batches each); on-core, 14 batches
group into 126-row tiles so per-batch 9x9 attention becomes block-diag
math driven by a host-precomputed mask.

v4 (vs the dual-load v2 baseline):
- Single input load: only the row-major xv [rows, P*C] shard is DMA'd
  (HBM traffic ~43MB/core instead of ~64MB). The channel-major xT
  layout needed by the fc matmul is built on-chip with PE transposes
  (PSUM) + vector-engine copies to SBUF.
- The second-order softmax is algebraically collapsed: with the -100
  diagonal self-mask, softmax(attn@attn/3) == I to beyond f64 precision,
  so attn2 is replaced by +1 on the diagonal folded into the mask.
- The attention matrix f = gram*scale + mask is SYMMETRIC, so with the
  softmax max-subtraction dropped (logits are O(10), exp is safe in
  f32), exp(f) is symmetric and serves directly as the lhsT of attn@v.
  No transpose of the attention weights; 1/rowsum is applied as a
  per-partition scale in the PSUM->SBUF output copies.
"""

import os
import numpy as np

PS = 3
N = 9           # patches per image
P = 9           # tokens per patch
B = 2048
C = 512
HID = 256
NCORES = 8
BLOC = B // NCORES          # 256 batches per core
ROWS = BLOC * N             # 2304 rows per core
G = 14                      # batches per group
GR = G * N                  # 126 rows per group
FULL_GROUPS = BLOC // G     # 18
REM = BLOC - FULL_GROUPS * G  # 4 remaining batches
SCALE = float((HID * P) ** -0.5)
NEG = -1.0e30

_CACHE = {}


def _groups():
    gs = [(g * G, G) for g in range(FULL_GROUPS)]
    if REM:
        gs.append((FULL_GROUPS * G, REM))
    return gs


def _build():
    import concourse.bacc as bacc
    import concourse.tile as tile
    from concourse import mybir

    BF = mybir.dt.bfloat16
    F32 = mybir.dt.float32
    I32 = mybir.dt.int32
    Copy = mybir.ActivationFunctionType.Copy
    Identity = mybir.ActivationFunctionType.Identity
    Exp = mybir.ActivationFunctionType.Exp
    MUL = mybir.AluOpType.mult
    ADD = mybir.AluOpType.add

    nc = bacc.Bacc("TRN2", target_bir_lowering=False)

    # xv rows padded on the host (+128 zero rows) so every group loads a
    # full 128-row tile (the PE transpose consumes 128 token columns).
    xv = nc.dram_tensor("xv", [ROWS + 128, P * C], BF, kind="ExternalInput")
    wT = nc.dram_tensor("wT", [C, HID], BF, kind="ExternalInput")
    b2 = nc.dram_tensor("b2", [128, 2], F32, kind="ExternalInput")
    m4 = nc.dram_tensor("m4", [GR, GR], F32, kind="ExternalInput")
    eye = nc.dram_tensor("eye", [128, 128], BF, kind="ExternalInput")
    out = nc.dram_tensor("out", [ROWS, P * C], BF, kind="ExternalOutput")

    groups = _groups()
    ng = len(groups)

    with tile.TileContext(nc) as tc:
        with (
            tc.tile_pool(name="const", bufs=1) as cpool,
            tc.tile_pool(name="xvp", bufs=6) as xvpool,
            tc.tile_pool(name="xtp", bufs=3) as xtpool,
            tc.tile_pool(name="xqp", bufs=2) as xqpool,
            tc.tile_pool(name="outp", bufs=3) as outpool,
            tc.tile_pool(name="small", bufs=3) as spool,
            tc.tile_pool(name="psfc", bufs=2, space="PSUM") as psfc_pool,
            tc.tile_pool(name="psg", bufs=2, space="PSUM") as psg_pool,
            tc.tile_pool(name="pso", bufs=2, space="PSUM") as pso_pool,
            tc.tile_pool(name="psx", bufs=2, space="PSUM") as psx_pool,
        ):
            wS = cpool.tile([128, 4 * HID], BF)
            for j in range(4):
                nc.sync.dma_start(
                    out=wS[:, j * HID:(j + 1) * HID],
                    in_=wT[j * 128:(j + 1) * 128, :],
                )
            bS = cpool.tile([128, 2], F32)
            nc.sync.dma_start(out=bS[:, :], in_=b2[:, :])
            m4S = cpool.tile([GR, GR], F32)
            nc.sync.dma_start(out=m4S[:, :], in_=m4[:, :])
            eyeS = cpool.tile([128, 128], BF)
            nc.sync.dma_start(out=eyeS[:, :], in_=eye[:, :])

            st = {}

            def emit_load(g):
                b0, gb = groups[g]
                r0 = b0 * N
                xvT = xvpool.tile([128, P * C], BF, tag="xv")
                # 3 chunks so the first transposes can start ~1/3 load in
                for ck in range(3):
                    c0 = ck * 3 * 512
                    nc.sync.dma_start(
                        out=xvT[:, c0:c0 + 3 * 512],
                        in_=xv[r0:r0 + 128, c0:c0 + 3 * 512],
                    )
                st[g] = {"xvT": xvT, "rows": gb * N, "r0": r0}

            def emit_xpose_start(g):
                # allocate the destination tile; units are emitted interleaved
                s = st[g]
                # xT4[c0, p, j, t] = xv[t, p*512 + j*128 + c0]
                s["xT4"] = xtpool.tile(
                    [128, P, 4, 128], BF, tag="xt4", name="xt4"
                )

            def emit_xpose_unit(g, p0, np_):
                # one unit: np_ (1 or 2) p-slices transposed into one PSUM
                # bank, then a single int32-view copy to SBUF
                s = st[g]
                xvT, xT4 = s["xvT"], s["xT4"]
                px = psx_pool.tile([128, 2, 4, 128], BF, tag="psx", name="px")
                for i in range(np_):
                    p = p0 + i
                    for j in range(4):
                        nc.tensor.transpose(
                            px[:, i, j],
                            xvT[:, p * 512 + j * 128:p * 512 + (j + 1) * 128],
                            eyeS[:, :],
                        )
                nc.vector.tensor_copy(
                    out=xT4[:, p0:p0 + np_].bitcast(I32),
                    in_=px[:, 0:np_].bitcast(I32),
                )

            def emit_fc_chunk(g, h, pc):
                s = st[g]
                xT4, xq = s["xT4"], s["xq"]
                ps = psfc_pool.tile([128, 3 * 128], F32, tag="psfc")
                for j in range(4):
                    nc.tensor.matmul(
                        ps[:, :],
                        lhsT=wS[:, j * HID + h * 128:j * HID + h * 128 + 128],
                        rhs=xT4[:, 3 * pc:3 * pc + 3, j, :],
                        start=(j == 0),
                        stop=(j == 3),
                    )
                nc.scalar.activation(
                    xq[:, h, 3 * pc:3 * pc + 3, :], ps[:, :],
                    Identity, bias=bS[:, h:h + 1], scale=1.0,
                )

            def emit_gram(g):
                s = st[g]
                rows = s["rows"]
                xq = s["xq"]
                psg = psg_pool.tile([GR, GR], F32, tag="psg")
                for h in range(2):
                    for p in range(P):
                        k = h * P + p
                        sl = xq[:, h, p, :rows]
                        nc.tensor.matmul(
                            psg[:rows, :rows],
                            lhsT=sl,
                            rhs=sl,
                            start=(k == 0),
                            stop=(k == 2 * P - 1),
                        )
                s["psg"] = psg

            def emit_tail_a(g):
                s = st[g]
                rows = s["rows"]
                psg = s["psg"]
                # f = gram*scale + mask; symmetric (gram and mask both are).
                f2 = spool.tile([128, 128], F32, tag="f2")
                nc.vector.scalar_tensor_tensor(
                    out=f2[:rows, :rows], in0=psg[:rows, :rows],
                    scalar=SCALE, in1=m4S[:rows, :rows], op0=MUL, op1=ADD,
                )
                # exp without max-subtraction: in-block logits are O(10),
                # exp stays far from f32/bf16 overflow; masked entries
                # underflow to exactly 0.  e2 is symmetric -> usable as lhsT.
                e2 = spool.tile([128, 128], BF, tag="e2")
                sm2 = spool.tile([128, 1], F32, tag="sm2")
                nc.scalar.activation(
                    e2[:rows, :rows], f2[:rows, :rows], Exp,
                    bias=0.0, scale=1.0, accum_out=sm2[:rows],
                )
                ri2 = spool.tile([128, 1], F32, tag="ri2")
                nc.vector.reciprocal(ri2[:rows], sm2[:rows])
                s["e2"] = e2
                s["ri2"] = ri2

            def emit_tail_b(g):
                s = st[g]
                rows, r0 = s["rows"], s["r0"]
                e2, ri2, xvT = s["e2"], s["ri2"], s["xvT"]
                outsb = outpool.tile([GR, P * C], BF, tag="outsb")
                for p in range(P):
                    pso = pso_pool.tile([GR, 512], F32, tag="pso")
                    nc.tensor.matmul(
                        pso[:rows, :],
                        lhsT=e2[:rows, :rows],
                        rhs=xvT[:rows, p * 512:(p + 1) * 512],
                        start=True, stop=True,
                    )
                    osl = outsb[:rows, p * 512:(p + 1) * 512]
                    if p % 2 == 0 and p < 8:
                        nc.scalar.activation(
                            osl, pso[:rows, :], Copy, scale=ri2[:rows],
                        )
                    else:
                        nc.vector.tensor_scalar_mul(
                            osl, pso[:rows, :], ri2[:rows],
                        )
                nc.gpsimd.dma_start(out=out[r0:r0 + rows, :], in_=outsb[:rows, :])
                del st[g]

            XP_UNITS = [(0, 2), (2, 2), (4, 2), (6, 2), (8, 1)]

            def emit_xpose_all(g):
                emit_xpose_start(g)
                for p0, np_ in XP_UNITS:
                    emit_xpose_unit(g, p0, np_)

            emit_load(0)
            emit_load(1)
            emit_xpose_all(0)
            emit_load(2)
            emit_xpose_all(1)
            emit_load(3)
            for g in range(ng):
                if g > 0:
                    emit_tail_a(g - 1)
                if g + 4 < ng:
                    emit_load(g + 4)
                # xq tile for g allocated up-front; fc chunks interleave with
                # the transposes of group g+2 so the PE queue never heads-of-
                # line-blocks on a psx buffer waiting for its DVE drain.
                st[g]["xq"] = xqpool.tile(
                    [128, 2, P, 128], BF, tag="xq", name="xq"
                )
                do_xp = g + 2 < ng
                if do_xp:
                    emit_xpose_start(g + 2)
                xp = 0
                for k, (h, pc) in enumerate(
                    [(h, pc) for h in range(2) for pc in range(3)]
                ):
                    if do_xp and xp < len(XP_UNITS):
                        emit_xpose_unit(g + 2, *XP_UNITS[xp])
                        xp += 1
                    emit_fc_chunk(g, h, pc)
                # attn@v of the previous group runs while the scalar engine
                # finishes this group's last xq copy, hiding that latency
                # before gram starts
                if g > 0:
                    emit_tail_b(g - 1)
                emit_gram(g)
            emit_tail_a(ng - 1)
            emit_tail_b(ng - 1)

    nc.finalize()
    return nc


def _host_prep(x, W_fc, b_fc):
    from concourse import mybir

    bf16 = mybir.dt.np(mybir.dt.bfloat16)
    # patch view: token order (b, n=(mi,mj), p=(pi,pj))
    xfc = x.reshape(B, PS, PS, PS, PS, C).transpose(0, 1, 3, 2, 4, 5)
    xfc = np.ascontiguousarray(xfc).reshape(B, N * P, C)

    blockmask = np.kron(np.eye(G, dtype=np.float32), np.ones((N, N), np.float32))
    # off-block -1e30; diagonal -99 = -100 (self mask) + 1 (attn2 == I)
    m4 = ((1.0 - blockmask) * NEG - 99.0 * np.eye(GR, dtype=np.float32)).astype(
        np.float32
    )
    eye = np.eye(128, dtype=np.float32).astype(bf16)
    wT = np.ascontiguousarray(W_fc.T).astype(bf16)           # [C, HID]
    b2 = np.ascontiguousarray(b_fc.reshape(2, 128).T).astype(np.float32)

    in_maps = []
    for i in range(NCORES):
        sh = xfc[i * BLOC:(i + 1) * BLOC].reshape(BLOC * N, P * C)
        xv_i = np.zeros((ROWS + 128, P * C), dtype=bf16)
        xv_i[:ROWS] = sh.astype(bf16)
        in_maps.append({"xv": xv_i, "wT": wT, "b2": b2, "m4": m4, "eye": eye})
    return in_maps


def kernel(x, W_fc, b_fc):
    from concourse.bass_utils import run_bass_kernel_spmd

    x = np.asarray(x, dtype=np.float32)
    W_fc = np.asarray(W_fc, dtype=np.float32)
    b_fc = np.asarray(b_fc, dtype=np.float32)

    if "nc" not in _CACHE:
        _CACHE["nc"] = _build()
    nc = _CACHE["nc"]
    in_maps = _host_prep(x, W_fc, b_fc)

    trace = bool(int(os.environ.get("KERNEL_TRACE", "0")))
    res = run_bass_kernel_spmd(
        nc, in_maps, core_ids=list(range(NCORES)), trace=trace
    )
    _CACHE["last_result"] = res

    outs = [np.asarray(r["out"], dtype=np.float32) for r in res.results]
    o = np.concatenate(outs, axis=0).reshape(B, PS, PS, PS, PS, C)
    o = o.transpose(0, 1, 3, 2, 4, 5).reshape(B, N, N, C)
    return np.ascontiguousarray(o)


# revision 31
# speedup vs baseline: 1.1435x; 1.0060x over previous
"""Trainium2 Bass kernel for nn_Attention_68015102099893 (sparse_attention).

B=2048 independent 9x9 attention blocks over patch tokens, fc 512->256.
Pure data parallel over 8 cores (256 batches each); on-core, 14 batches
group into 126-row tiles so per-batch 9x9 attention becomes block-diag
math driven by a host-precomputed mask.

v4 (vs the dual-load v2 baseline):
- Single input load: only the row-major xv [rows, P*C] shard is DMA'd
  (HBM traffic ~43MB/core instead of ~64MB). The channel-major xT
  layout needed by the fc matmul is built on-chip with PE transposes
  (PSUM) + vector-engine copies to SBUF.
- The second-order softmax is algebraically collapsed: with the -100
  diagonal self-mask, softmax(attn@attn/3) == I to beyond f64 precision,
  so attn2 is replaced by +1 on the diagonal folded into the mask.
- The attention matrix f = gram*scale + mask is SYMMETRIC, so with the
  softmax max-subtraction dropped (logits are O(10), exp is safe in
  f32), exp(f) is symmetric and serves directly as the lhsT of attn@v.
  No transpose of the attention weights; 1/rowsum is applied as a
  per-partition scale in the PSUM->SBUF output copies.
"""

import os
import numpy as np

PS = 3
N = 9           # patches per image
P = 9           # tokens per patch
B = 2048
C = 512
HID = 256
NCORES = 8
BLOC = B // NCORES          # 256 batches per core
ROWS = BLOC * N             # 2304 rows per core
G = 14                      # batches per group
GR = G * N                  # 126 rows per group
FULL_GROUPS = BLOC // G     # 18
REM = BLOC - FULL_GROUPS * G  # 4 remaining batches
SCALE = float((HID * P) ** -0.5)
NEG = -1.0e30

_CACHE = {}


def _groups():
    gs = [(g * G, G) for g in range(FULL_GROUPS)]
    if REM:
        gs.append((FULL_GROUPS * G, REM))
    return gs


def _build():
    import concourse.bacc as bacc
    import concourse.tile as tile
    from concourse import mybir

    BF = mybir.dt.bfloat16
    F32 = mybir.dt.float32
    I32 = mybir.dt.int32
    Copy = mybir.ActivationFunctionType.Copy
    Identity = mybir.ActivationFunctionType.Identity
    Exp = mybir.ActivationFunctionType.Exp
    MUL = mybir.AluOpType.mult
    ADD = mybir.AluOpType.add

    nc = bacc.Bacc("TRN2", target_bir_lowering=False)

    # xv rows padded on the host (+128 zero rows) so every group loads a
    # full 128-row tile (the PE transpose consumes 128 token columns).
    xv = nc.dram_tensor("xv", [ROWS + 128, P * C], BF, kind="ExternalInput")
    wT = nc.dram_tensor("wT", [C, HID], BF, kind="ExternalInput")
    b2 = nc.dram_tensor("b2", [128, 2], F32, kind="ExternalInput")
    m4 = nc.dram_tensor("m4", [GR, GR], F32, kind="ExternalInput")
    eye = nc.dram_tensor("eye", [128, 128], BF, kind="ExternalInput")
    out = nc.dram_tensor("out", [ROWS, P * C], BF, kind="ExternalOutput")

    groups = _groups()
    ng = len(groups)

    with tile.TileContext(nc) as tc:
        with (
            tc.tile_pool(name="const", bufs=1) as cpool,
            tc.tile_pool(name="xvp", bufs=6) as xvpool,
            tc.tile_pool(name="xtp", bufs=3) as xtpool,
            tc.tile_pool(name="xqp", bufs=2) as xqpool,
            tc.tile_pool(name="outp", bufs=3) as outpool,
            tc.tile_pool(name="small", bufs=3) as spool,
            tc.tile_pool(name="psfc", bufs=2, space="PSUM") as psfc_pool,
            tc.tile_pool(name="psg", bufs=2, space="PSUM") as psg_pool,
            tc.tile_pool(name="pso", bufs=2, space="PSUM") as pso_pool,
            tc.tile_pool(name="psx", bufs=2, space="PSUM") as psx_pool,
        ):
            # eye goes first on the sync queue (the very first PE transposes
            # need it); bulk consts go on the scalar hwdge queue so they
            # don't delay load(0)
            eyeS = cpool.tile([128, 128], BF)
            nc.sync.dma_start(out=eyeS[:, :], in_=eye[:, :])
            wS = cpool.tile([128, 4 * HID], BF)
            for j in range(4):
                nc.scalar.dma_start(
                    out=wS[:, j * HID:(j + 1) * HID],
                    in_=wT[j * 128:(j + 1) * 128, :],
                )
            bS = cpool.tile([128, 2], F32)
            nc.scalar.dma_start(out=bS[:, :], in_=b2[:, :])
            m4S = cpool.tile([GR, GR], F32)
            nc.scalar.dma_start(out=m4S[:, :], in_=m4[:, :])

            st = {}

            def emit_load(g):
                b0, gb = groups[g]
                r0 = b0 * N
                xvT = xvpool.tile([128, P * C], BF, tag="xv")
                # 3 chunks so the first transposes can start ~1/3 load in
                for ck in range(3):
                    c0 = ck * 3 * 512
                    nc.sync.dma_start(
                        out=xvT[:, c0:c0 + 3 * 512],
                        in_=xv[r0:r0 + 128, c0:c0 + 3 * 512],
                    )
                st[g] = {"xvT": xvT, "rows": gb * N, "r0": r0}

            def emit_xpose_start(g):
                # allocate the destination tile; units are emitted interleaved
                s = st[g]
                # xT4[c0, p, j, t] = xv[t, p*512 + j*128 + c0]
                s["xT4"] = xtpool.tile(
                    [128, P, 4, 128], BF, tag="xt4", name="xt4"
                )

            def emit_xpose_unit(g, p0, np_):
                # one unit: np_ (1 or 2) p-slices transposed into one PSUM
                # bank, then a single int32-view copy to SBUF
                s = st[g]
                xvT, xT4 = s["xvT"], s["xT4"]
                px = psx_pool.tile([128, 2, 4, 128], BF, tag="psx", name="px")
                for i in range(np_):
                    p = p0 + i
                    for j in range(4):
                        nc.tensor.transpose(
                            px[:, i, j],
                            xvT[:, p * 512 + j * 128:p * 512 + (j + 1) * 128],
                            eyeS[:, :],
                        )
                nc.vector.tensor_copy(
                    out=xT4[:, p0:p0 + np_].bitcast(I32),
                    in_=px[:, 0:np_].bitcast(I32),
                )

            def emit_fc_chunk(g, h, pc):
                s = st[g]
                xT4, xq = s["xT4"], s["xq"]
                ps = psfc_pool.tile([128, 3 * 128], F32, tag="psfc")
                for j in range(4):
                    nc.tensor.matmul(
                        ps[:, :],
                        lhsT=wS[:, j * HID + h * 128:j * HID + h * 128 + 128],
                        rhs=xT4[:, 3 * pc:3 * pc + 3, j, :],
                        start=(j == 0),
                        stop=(j == 3),
                    )
                nc.scalar.activation(
                    xq[:, h, 3 * pc:3 * pc + 3, :], ps[:, :],
                    Identity, bias=bS[:, h:h + 1], scale=1.0,
                )

            def emit_gram(g):
                s = st[g]
                rows = s["rows"]
                xq = s["xq"]
                psg = psg_pool.tile([GR, GR], F32, tag="psg")
                for h in range(2):
                    for p in range(P):
                        k = h * P + p
                        sl = xq[:, h, p, :rows]
                        nc.tensor.matmul(
                            psg[:rows, :rows],
                            lhsT=sl,
                            rhs=sl,
                            start=(k == 0),
                            stop=(k == 2 * P - 1),
                        )
                s["psg"] = psg

            def emit_tail_a(g):
                s = st[g]
                rows = s["rows"]
                psg = s["psg"]
                # f = gram*scale + mask; symmetric (gram and mask both are).
                f2 = spool.tile([128, 128], F32, tag="f2")
                nc.vector.scalar_tensor_tensor(
                    out=f2[:rows, :rows], in0=psg[:rows, :rows],
                    scalar=SCALE, in1=m4S[:rows, :rows], op0=MUL, op1=ADD,
                )
                # exp without max-subtraction: in-block logits are O(10),
                # exp stays far from f32/bf16 overflow; masked entries
                # underflow to exactly 0.  e2 is symmetric -> usable as lhsT.
                e2 = spool.tile([128, 128], BF, tag="e2")
                sm2 = spool.tile([128, 1], F32, tag="sm2")
                nc.scalar.activation(
                    e2[:rows, :rows], f2[:rows, :rows], Exp,
                    bias=0.0, scale=1.0, accum_out=sm2[:rows],
                )
                ri2 = spool.tile([128, 1], F32, tag="ri2")
                nc.vector.reciprocal(ri2[:rows], sm2[:rows])
                s["e2"] = e2
                s["ri2"] = ri2

            def emit_tail_b(g):
                s = st[g]
                rows, r0 = s["rows"], s["r0"]
                e2, ri2, xvT = s["e2"], s["ri2"], s["xvT"]
                outsb = outpool.tile([GR, P * C], BF, tag="outsb")
                for p in range(P):
                    pso = pso_pool.tile([GR, 512], F32, tag="pso")
                    nc.tensor.matmul(
                        pso[:rows, :],
                        lhsT=e2[:rows, :rows],
                        rhs=xvT[:rows, p * 512:(p + 1) * 512],
                        start=True, stop=True,
                    )
                    osl = outsb[:rows, p * 512:(p + 1) * 512]
                    if p % 2 == 0 and p < 8:
                        nc.scalar.activation(
                            osl, pso[:rows, :], Copy, scale=ri2[:rows],
                        )
                    else:
                        nc.vector.tensor_scalar_mul(
                            osl, pso[:rows, :], ri2[:rows],
                        )
                    if p in (2, 5):
                        c0 = (p - 2) * 512
                        nc.gpsimd.dma_start(
                            out=out[r0:r0 + rows, c0:c0 + 3 * 512],
                            in_=outsb[:rows, c0:c0 + 3 * 512],
                        )
                nc.gpsimd.dma_start(
                    out=out[r0:r0 + rows, 6 * 512:],
                    in_=outsb[:rows, 6 * 512:],
                )
                del st[g]

            XP_UNITS = [(0, 2), (2, 2), (4, 2), (6, 2), (8, 1)]

            def emit_xpose_all(g):
                emit_xpose_start(g)
                for p0, np_ in XP_UNITS:
                    emit_xpose_unit(g, p0, np_)

            emit_load(0)
            emit_load(1)
            emit_xpose_all(0)
            emit_load(2)
            emit_xpose_all(1)
            emit_load(3)
            for g in range(ng):
                if g > 0:
                    emit_tail_a(g - 1)
                if g + 4 < ng:
                    emit_load(g + 4)
                # xq tile for g allocated up-front; fc chunks interleave with
                # the transposes of group g+2 so the PE queue never heads-of-
                # line-blocks on a psx buffer waiting for its DVE drain.
                st[g]["xq"] = xqpool.tile(
                    [128, 2, P, 128], BF, tag="xq", name="xq"
                )
                do_xp = g + 2 < ng
                if do_xp:
                    emit_xpose_start(g + 2)
                xp = 0
                for k, (h, pc) in enumerate(
                    [(h, pc) for h in range(2) for pc in range(3)]
                ):
                    if do_xp and xp < len(XP_UNITS):
                        emit_xpose_unit(g + 2, *XP_UNITS[xp])
                        xp += 1
                    emit_fc_chunk(g, h, pc)
                # attn@v of the previous group runs while the scalar engine
                # finishes this group's last xq copy, hiding that latency
                # before gram starts
                if g > 0:
                    emit_tail_b(g - 1)
                emit_gram(g)
            emit_tail_a(ng - 1)
            emit_tail_b(ng - 1)

    nc.finalize()
    return nc


def _host_prep(x, W_fc, b_fc):
    from concourse import mybir

    bf16 = mybir.dt.np(mybir.dt.bfloat16)
    # patch view: token order (b, n=(mi,mj), p=(pi,pj))
    xfc = x.reshape(B, PS, PS, PS, PS, C).transpose(0, 1, 3, 2, 4, 5)
    xfc = np.ascontiguousarray(xfc).reshape(B, N * P, C)

    blockmask = np.kron(np.eye(G, dtype=np.float32), np.ones((N, N), np.float32))
    # off-block -1e30; diagonal -99 = -100 (self mask) + 1 (attn2 == I)
    m4 = ((1.0 - blockmask) * NEG - 99.0 * np.eye(GR, dtype=np.float32)).astype(
        np.float32
    )
    eye = np.eye(128, dtype=np.float32).astype(bf16)
    wT = np.ascontiguousarray(W_fc.T).astype(bf16)           # [C, HID]
    b2 = np.ascontiguousarray(b_fc.reshape(2, 128).T).astype(np.float32)

    in_maps = []
    for i in range(NCORES):
        sh = xfc[i * BLOC:(i + 1) * BLOC].reshape(BLOC * N, P * C)
        xv_i = np.zeros((ROWS + 128, P * C), dtype=bf16)
        xv_i[:ROWS] = sh.astype(bf16)
        in_maps.append({"xv": xv_i, "wT": wT, "b2": b2, "m4": m4, "eye": eye})
    return in_maps


def kernel(x, W_fc, b_fc):
    from concourse.bass_utils import run_bass_kernel_spmd

    x = np.asarray(x, dtype=np.float32)
    W_fc = np.asarray(W_fc, dtype=np.float32)
    b_fc = np.asarray(b_fc, dtype=np.float32)

    if "nc" not in _CACHE:
        _CACHE["nc"] = _build()
    nc = _CACHE["nc"]
    in_maps = _host_prep(x, W_fc, b_fc)

    trace = bool(int(os.environ.get("KERNEL_TRACE", "0")))
    res = run_bass_kernel_spmd(
        nc, in_maps, core_ids=list(range(NCORES)), trace=trace
    )
    _CACHE["last_result"] = res

    outs = [np.asarray(r["out"], dtype=np.float32) for r in res.results]
    o = np.concatenate(outs, axis=0).reshape(B, PS, PS, PS, PS, C)
    o = o.transpose(0, 1, 3, 2, 4, 5).reshape(B, N, N, C)
    return np.ascontiguousarray(o)
